# revision 5
# baseline (speedup 1.0000x reference)
"""Trainium2 Bass kernel for DequantingLinear (GGML Q8_0 dequant + linear).

Computes out[4096, 12288] = x[4096, 3072] @ dequant(w_q, w_scales).T + bias
where w_q is int32 (int8-valued) with per-32-element-block fp32 scales.

Sharding: tensor-parallel over output features across 8 NeuronCores. Each
core gets the full x and a 1536-row shard of w_q / w_scales / bias,
computes its [4096, 1536] output slice; the host concatenates on axis 1.

Per-core kernel (Tile framework):
  * w path: int32 row-tiles load on the SP-HWDGE ring, are dequantized in a
    single mixed-dtype vector-engine multiply (int32 x block-broadcast fp32
    -> bf16), bounced through DRAM, and xbar-transpose-loaded into a
    resident [in, k, out] SBUF tensor. (Matmul needs the contraction dim on
    partitions; the xbar transpose is 2-byte only, hence the bf16 bounce.)
  * x path: fp32 -> bf16 DRAM->DRAM SWDGE casts into a 4-slot ring of
    internal DRAM tensors, then per-block xbar-transpose loads to
    [in, k, tok]. Casts beyond the first two carry an explicit pacing
    dependency on w-prep completion so the bulk x traffic cannot starve
    the w chain that gates every matmul.
  * GEMM: psum[128 tok, 512 out] tiles accumulate 24 bf16 k-tile matmuls
    (fp32 PSUM); bias is added during the PSUM->SBUF copy on the vector
    engine; results stream out.
  All HWDGE DMAs are issued on nc.sync — ACT-issued DMAs were observed to
  produce corrupted results on hardware in this configuration.
"""

import sys

for _p in ("/opt/trn_rl_repo",):
    if _p not in sys.path:
        sys.path.append(_p)

from contextlib import ExitStack

import numpy as np

import concourse.bacc as bacc
import concourse.bass as bass
import concourse.mybir as mybir
from concourse import tile
from concourse.tile_rust import add_dep_helper
from concourse.bass_utils import run_bass_kernel_spmd

FP32 = mybir.dt.float32
BF16 = mybir.dt.bfloat16
INT32 = mybir.dt.int32

N_CORES = 8
TOK, IN, OUT = 4096, 3072, 12288
QK = 32
OUT_SH = OUT // N_CORES
TOK_BLK = 512
NCOL = 512
RING_SLOTS = 4
FREE_CASTS = 2


def _build(nc: bass.Bass):
    P = 128
    KT = IN // P
    NBLK = TOK // TOK_BLK
    MT = TOK_BLK // P
    NT = OUT_SH // NCOL
    NB = IN // QK
    OT = OUT_SH // P

    x = nc.dram_tensor("x", [TOK, IN], FP32, kind="ExternalInput")
    w_q = nc.dram_tensor("w_q", [OUT_SH, IN], INT32, kind="ExternalInput")
    w_scales = nc.dram_tensor("w_scales", [OUT_SH, NB], FP32, kind="ExternalInput")
    bias = nc.dram_tensor("bias", [OUT_SH], FP32, kind="ExternalInput")
    out = nc.dram_tensor("out", [TOK, OUT_SH], FP32, kind="ExternalOutput")

    x_slots = [
        nc.dram_tensor(f"x_bf16_{s}", [TOK_BLK, IN], BF16) for s in range(RING_SLOTS)
    ]
    w_bf16 = nc.dram_tensor("w_bf16", [OUT_SH, IN], BF16)

    with tile.TileContext(nc) as tc, ExitStack() as ctx:
        const_pool = ctx.enter_context(tc.tile_pool(name="const", bufs=1))
        wq_pool = ctx.enter_context(tc.tile_pool(name="wq", bufs=3))
        wd_pool = ctx.enter_context(tc.tile_pool(name="wd", bufs=3))
        wt_pool = ctx.enter_context(tc.tile_pool(name="wt", bufs=1))
        xt_pool = ctx.enter_context(tc.tile_pool(name="xt", bufs=2))
        out_pool = ctx.enter_context(tc.tile_pool(name="out", bufs=4))
        psum_pool = ctx.enter_context(tc.tile_pool(name="psum", bufs=8, space="PSUM"))

        bias_rep = const_pool.tile([P, OUT_SH], FP32, tag="bias_rep")
        nc.sync.dma_start(bias_rep[:], bias.ap().unsqueeze(0).to_broadcast([P, OUT_SH]))

        def cast_block(b):
            s = b % RING_SLOTS
            srows = slice(b * TOK_BLK, (b + 1) * TOK_BLK)
            return nc.gpsimd.dma_start(x_slots[s].ap()[:, :], x.ap()[srows, :])

        for b in range(min(FREE_CASTS, NBLK)):
            cast_block(b)

        # --- w-prep: load + dequant row-tiles, store bf16, transpose-load ---
        wt = wt_pool.tile([P, KT, OUT_SH], BF16, tag="wt")
        for o in range(OT):
            rows = slice(o * P, (o + 1) * P)
            wq_i = wq_pool.tile([P, IN], INT32, tag="wq")
            nc.sync.dma_start(wq_i[:], w_q.ap()[rows, :])
            sc = wq_pool.tile([P, NB], FP32, tag="sc")
            nc.sync.dma_start(sc[:], w_scales.ap()[rows, :])
            wd = wd_pool.tile([P, IN], BF16, tag="wd")
            nc.vector.tensor_mul(
                wd[:].rearrange("p (b q) -> p b q", q=QK),
                wq_i[:].rearrange("p (b q) -> p b q", q=QK),
                sc[:].unsqueeze(2).to_broadcast([P, NB, QK]),
            )
            nc.sync.dma_start(w_bf16.ap()[rows, :], wd[:])
        last_xbar = None
        for k in range(KT):
            last_xbar = nc.sync.dma_start(
                wt[:, k, :], w_bf16.ap()[:, k * P : (k + 1) * P], transpose=True
            )

        # --- main loop over token blocks ---
        for b in range(NBLK):
            s = b % RING_SLOTS
            xt = xt_pool.tile([P, KT, TOK_BLK], BF16, tag="xt")
            for k in range(KT):
                nc.sync.dma_start(
                    xt[:, k, :],
                    x_slots[s].ap()[:, k * P : (k + 1) * P],
                    transpose=True,
                )
            bnext = b + FREE_CASTS
            if bnext < NBLK:
                ci = cast_block(bnext)
                add_dep_helper(
                    ci.ins, last_xbar.ins, reason="pace x casts behind w-prep"
                )
            for m in range(MT):
                tok0 = b * TOK_BLK + m * P
                for n in range(NT):
                    ps = psum_pool.tile([P, NCOL], FP32, tag="ps")
                    for k in range(KT):
                        nc.tensor.matmul(
                            ps[:],
                            xt[:, k, m * P : (m + 1) * P],
                            wt[:, k, n * NCOL : (n + 1) * NCOL],
                            start=(k == 0),
                            stop=(k == KT - 1),
                        )
                    ob = out_pool.tile([P, NCOL], FP32, tag="ob")
                    nc.vector.tensor_add(
                        ob[:], ps[:], bias_rep[:, n * NCOL : (n + 1) * NCOL]
                    )
                    nc.sync.dma_start(
                        out.ap()[tok0 : tok0 + P, n * NCOL : (n + 1) * NCOL], ob[:]
                    )
    return nc


_COMPILED_NC = None


def _get_nc():
    global _COMPILED_NC
    if _COMPILED_NC is None:
        nc = bacc.Bacc("TRN2", target_bir_lowering=False, debug=False)
        _build(nc)
        nc.compile()
        _COMPILED_NC = nc
    return _COMPILED_NC


def kernel(x, w_q, w_scales, bias):
    assert x.shape == (TOK, IN) and w_q.shape == (OUT, IN)
    nc = _get_nc()
    x = np.ascontiguousarray(np.asarray(x, dtype=np.float32))
    w_q = np.asarray(w_q, dtype=np.int32)
    w_scales = np.asarray(w_scales, dtype=np.float32)
    bias = np.asarray(bias, dtype=np.float32)
    in_maps = []
    for c in range(N_CORES):
        r = slice(c * OUT_SH, (c + 1) * OUT_SH)
        in_maps.append(
            {
                "x": x,
                "w_q": np.ascontiguousarray(w_q[r]),
                "w_scales": np.ascontiguousarray(w_scales[r]),
                "bias": np.ascontiguousarray(bias[r]),
            }
        )
    res = run_bass_kernel_spmd(nc, in_maps, list(range(N_CORES)))
    return np.concatenate([res.results[c]["out"] for c in range(N_CORES)], axis=1)


# revision 6
# speedup vs baseline: 1.8236x; 1.8236x over previous
"""Trainium2 Bass kernel for DequantingLinear (GGML Q8_0 dequant + linear).

Computes out[4096, 12288] = x[4096, 3072] @ dequant(w_q, w_scales).T + bias
where w_q is int32 (int8-valued) with per-32-element-block fp32 scales.

Sharding: tensor-parallel over output features across 8 NeuronCores. Each
core gets the full x and a 1536-row shard of w_q / w_scales / bias,
computes its [4096, 1536] output slice; the host concatenates on axis 1.

Per-core kernel (Tile framework):
  * w path: w-prep is chunked by OUTPUT ROWS (512-out row-chunks): each
    chunk needs only 4 dequants (one mixed-dtype vector multiply each:
    int32 x block-broadcast fp32 scales -> bf16, exact for |q|<=127), a
    DRAM bounce, and 24 xbar-transpose loads into the resident
    [in, k, out] SBUF weight tensor. A phase-1 GEMM (n=0 output columns,
    first two token blocks, xt preloaded) runs as soon as row-chunk 0
    lands, filling the pipeline head while later chunks stream.
  * x path: fp32 -> bf16 DRAM->DRAM SWDGE casts into a 4-slot ring of
    internal DRAM tensors, then per-block xbar-transpose loads to
    [in, k, tok]. Casts beyond the first two carry an explicit pacing
    dependency on w-prep completion so bulk x traffic cannot starve the
    w chain.
  * GEMM: psum[128 tok, 512 out] tiles accumulate 24 bf16 k-tile matmuls
    (fp32 PSUM); bias is added during the PSUM->SBUF copy on the vector
    engine.
  All HWDGE DMAs are issued on nc.sync — ACT-issued DMAs were observed to
  produce corrupted results on hardware in this configuration.
"""

import sys

for _p in ("/opt/trn_rl_repo",):
    if _p not in sys.path:
        sys.path.append(_p)

from contextlib import ExitStack

import numpy as np

import concourse.bacc as bacc
import concourse.bass as bass
import concourse.mybir as mybir
from concourse import tile
from concourse.tile_rust import add_dep_helper
from concourse.bass_utils import run_bass_kernel_spmd

FP32 = mybir.dt.float32
BF16 = mybir.dt.bfloat16
INT32 = mybir.dt.int32

N_CORES = 8
TOK, IN, OUT = 4096, 3072, 12288
QK = 32
OUT_SH = OUT // N_CORES
TOK_BLK = 512
NCOL = 512
RING_SLOTS = 4
FREE_CASTS = 2
NB1 = 2


def _build(nc: bass.Bass):
    P = 128
    KT = IN // P
    NBLK = TOK // TOK_BLK
    MT = TOK_BLK // P
    NT = OUT_SH // NCOL
    NB = IN // QK
    OT = OUT_SH // P
    RCOT = NCOL // P

    x = nc.dram_tensor("x", [TOK, IN], FP32, kind="ExternalInput")
    w_q = nc.dram_tensor("w_q", [OUT_SH, IN], INT32, kind="ExternalInput")
    w_scales = nc.dram_tensor("w_scales", [OUT_SH, NB], FP32, kind="ExternalInput")
    bias = nc.dram_tensor("bias", [OUT_SH], FP32, kind="ExternalInput")
    out = nc.dram_tensor("out", [TOK, OUT_SH], FP32, kind="ExternalOutput")

    x_slots = [
        nc.dram_tensor(f"x_bf16_{s}", [TOK_BLK, IN], BF16) for s in range(RING_SLOTS)
    ]
    w_bf16 = nc.dram_tensor("w_bf16", [OUT_SH, IN], BF16)

    with tile.TileContext(nc) as tc, ExitStack() as ctx:
        const_pool = ctx.enter_context(tc.tile_pool(name="const", bufs=1))
        wq_pool = ctx.enter_context(tc.tile_pool(name="wq", bufs=3))
        wd_pool = ctx.enter_context(tc.tile_pool(name="wd", bufs=3))
        wt_pool = ctx.enter_context(tc.tile_pool(name="wt", bufs=1))
        xt_pool = ctx.enter_context(tc.tile_pool(name="xt", bufs=2))
        out_pool = ctx.enter_context(tc.tile_pool(name="out", bufs=4))
        psum_pool = ctx.enter_context(tc.tile_pool(name="psum", bufs=8, space="PSUM"))

        bias_rep = const_pool.tile([P, OUT_SH], FP32, tag="bias_rep")
        nc.sync.dma_start(bias_rep[:], bias.ap().unsqueeze(0).to_broadcast([P, OUT_SH]))

        def cast_block(b):
            s = b % RING_SLOTS
            srows = slice(b * TOK_BLK, (b + 1) * TOK_BLK)
            return nc.gpsimd.dma_start(x_slots[s].ap()[:, :], x.ap()[srows, :])

        for b in range(min(FREE_CASTS, NBLK)):
            cast_block(b)

        def load_xt(b):
            s = b % RING_SLOTS
            xt = xt_pool.tile([P, KT, TOK_BLK], BF16, tag="xt")
            for k in range(KT):
                nc.sync.dma_start(
                    xt[:, k, :],
                    x_slots[s].ap()[:, k * P : (k + 1) * P],
                    transpose=True,
                )
            return xt

        xt_cache = {}
        for b in range(NB1):
            xt_cache[b] = load_xt(b)

        sc_tiles = []
        for o in range(OT):
            sct = const_pool.tile([P, NB], FP32, tag=f"sc_{o}")
            nc.sync.dma_start(sct[:], w_scales.ap()[o * P : (o + 1) * P, :])
            sc_tiles.append(sct)

        wt = wt_pool.tile([P, KT, OUT_SH], BF16, tag="wt")
        last_xbar = None
        for rc in range(NT):
            for oo in range(RCOT):
                o = rc * RCOT + oo
                rows = slice(o * P, (o + 1) * P)
                wq_i = wq_pool.tile([P, IN], INT32, tag="wq")
                nc.sync.dma_start(wq_i[:], w_q.ap()[rows, :])
                wd = wd_pool.tile([P, IN], BF16, tag="wd")
                nc.vector.tensor_mul(
                    wd[:].rearrange("p (b q) -> p b q", q=QK),
                    wq_i[:].rearrange("p (b q) -> p b q", q=QK),
                    sc_tiles[o][:].unsqueeze(2).to_broadcast([P, NB, QK]),
                )
                nc.sync.dma_start(w_bf16.ap()[rows, :], wd[:])
            for k in range(KT):
                last_xbar = nc.sync.dma_start(
                    wt[:, k, rc * NCOL : (rc + 1) * NCOL],
                    w_bf16.ap()[rc * NCOL : (rc + 1) * NCOL, k * P : (k + 1) * P],
                    transpose=True,
                )

        def gemm_group(xt, b, m, n):
            tok0 = b * TOK_BLK + m * P
            ps = psum_pool.tile([P, NCOL], FP32, tag="ps")
            for k in range(KT):
                nc.tensor.matmul(
                    ps[:],
                    xt[:, k, m * P : (m + 1) * P],
                    wt[:, k, n * NCOL : (n + 1) * NCOL],
                    start=(k == 0),
                    stop=(k == KT - 1),
                )
            ob = out_pool.tile([P, NCOL], FP32, tag="ob")
            nc.vector.tensor_add(ob[:], ps[:], bias_rep[:, n * NCOL : (n + 1) * NCOL])
            nc.sync.dma_start(
                out.ap()[tok0 : tok0 + P, n * NCOL : (n + 1) * NCOL], ob[:]
            )

        for b in range(NB1):
            for m in range(MT):
                gemm_group(xt_cache[b], b, m, 0)

        ncast = min(FREE_CASTS, NBLK)
        for b in range(NBLK):
            if b in xt_cache:
                xt = xt_cache.pop(b)
            else:
                xt = load_xt(b)
            if ncast < NBLK:
                ci = cast_block(ncast)
                add_dep_helper(
                    ci.ins, last_xbar.ins, reason="pace x casts behind w-prep"
                )
                ncast += 1
            for m in range(MT):
                for n in range(NT):
                    if b < NB1 and n == 0:
                        continue
                    gemm_group(xt, b, m, n)
    return nc


_COMPILED_NC = None


def _get_nc():
    global _COMPILED_NC
    if _COMPILED_NC is None:
        nc = bacc.Bacc("TRN2", target_bir_lowering=False, debug=False)
        _build(nc)
        nc.compile()
        _COMPILED_NC = nc
    return _COMPILED_NC


def kernel(x, w_q, w_scales, bias):
    assert x.shape == (TOK, IN) and w_q.shape == (OUT, IN)
    nc = _get_nc()
    x = np.ascontiguousarray(np.asarray(x, dtype=np.float32))
    w_q = np.asarray(w_q, dtype=np.int32)
    w_scales = np.asarray(w_scales, dtype=np.float32)
    bias = np.asarray(bias, dtype=np.float32)
    in_maps = []
    for c in range(N_CORES):
        r = slice(c * OUT_SH, (c + 1) * OUT_SH)
        in_maps.append(
            {
                "x": x,
                "w_q": np.ascontiguousarray(w_q[r]),
                "w_scales": np.ascontiguousarray(w_scales[r]),
                "bias": np.ascontiguousarray(bias[r]),
            }
        )
    res = run_bass_kernel_spmd(nc, in_maps, list(range(N_CORES)))
    return np.concatenate([res.results[c]["out"] for c in range(N_CORES)], axis=1)
